# revision 9
# baseline (speedup 1.0000x reference)
"""Trainium2 Bass kernel for nn_ActorCriticGNN (3-layer GAT + actor/critic heads).

Strategy: dst-sharded edge parallelism over 8 NeuronCores.
 - Host sorts edges by dst; core c owns dst nodes [c*N/8, (c+1)*N/8) and all
   their incoming edges; per-dst segment softmax is then fully core-local.
 - Per layer: dma_gather packed rows [x | es] of edge sources from a local
   full-table slab; w = exp(leakyrelu(es[src]+ed[dst])) (softmax max-shift
   skipped -- it cancels exactly); messages aggregated with one-hot scatter
   matmuls into PSUM (one-hots host-precomputed, shared by all 3 layers).
   GAT in factored form: agg[n,h,:] = sum_e w_e^h x[src_e]; out=(agg@W_h)/den+b.
 - AllGather of the per-core output slab between layers.
 - Mean-pool via host-weighted one-hot matmul; MLP heads replicated per-core
   in transposed layout (biases per-partition -> free in ACT).
"""

import sys

sys.path.insert(0, "/root/.axon_site/_ro/trn_rl_repo")

import numpy as np
import ml_dtypes

import concourse.bacc as bacc
import concourse.bass as bass  # noqa
import concourse.mybir as mybir
import concourse.tile as tile
from concourse import library_config

BF16 = mybir.dt.bfloat16
F32 = mybir.dt.float32
I16 = mybir.dt.int16
AF = mybir.ActivationFunctionType

HH = 4
NEG = 0.2


def _bf(a):
    return np.asarray(a, dtype=ml_dtypes.bfloat16)


def _f32(a):
    return np.ascontiguousarray(a, dtype=np.float32)


# --------------------------------------------------------------------------
# Host preprocessing
# --------------------------------------------------------------------------
def preprocess(inputs, R=8):
    x = _f32(inputs["x"])
    ei = np.asarray(inputs["edge_index"])
    batch = np.asarray(inputs["batch"]).astype(np.int64)
    N = x.shape[0]
    E = ei.shape[1]
    src = ei[0].astype(np.int64)
    dst = ei[1].astype(np.int64)

    NPC = N // R
    assert NPC * R == N
    NPAD = ((NPC + 127) // 128) * 128
    CH = NPAD // 128
    RN = R * NPAD

    C1 = np.asarray(inputs["W1"]).shape[1] // HH
    C2 = np.asarray(inputs["W2"]).shape[1] // HH
    C3 = np.asarray(inputs["W3"]).shape[1] // HH
    XW1 = 16
    XW2, XW3 = HH * C1, HH * C2
    RW1 = 128
    RW2 = ((XW2 + HH + 127) // 128) * 128
    RW3 = ((XW3 + HH + 127) // 128) * 128

    # ---- edge partitioning ----
    core_of = dst // NPC
    rel = dst - core_of * NPC
    chunk_of = rel // 128
    nrel = rel - chunk_of * 128
    counts = np.zeros((R, CH), dtype=np.int64)
    np.add.at(counts, (core_of, chunk_of), 1)
    T = int(max(1, (counts.max() + 127) // 128))

    order = np.lexsort((chunk_of, core_of))
    so, nro = src[order], nrel[order]
    gidx_all = (so // NPC) * NPAD + (so % NPC)

    eidx = np.zeros((R, CH, T * 128), dtype=np.int64)
    nrel_p = np.full((R, CH, T * 128), -1, dtype=np.int64)
    pos = 0
    for c in range(R):
        for ch in range(CH):
            n = int(counts[c, ch])
            eidx[c, ch, :n] = gidx_all[pos : pos + n]
            nrel_p[c, ch, :n] = nro[pos : pos + n]
            pos += n
    assert pos == E

    IW = T * 128 // 16
    ei16 = np.zeros((R, CH, 128, IW), dtype=np.int16)
    arr = eidx.reshape(R, CH, IW, 16).astype(np.int16)
    for p in range(128):
        ei16[:, :, p, :] = arr[:, :, :, p % 16]

    S = np.zeros((R, CH, 128, T * 128), dtype=ml_dtypes.bfloat16)
    ST = np.zeros((R, CH, 128, T * 128), dtype=ml_dtypes.bfloat16)
    r4 = nrel_p.reshape(R, CH, T, 128)
    idx_r, idx_ch, idx_t, idx_e = np.nonzero(r4 >= 0)
    nvals = r4[idx_r, idx_ch, idx_t, idx_e]
    S[idx_r, idx_ch, idx_e, idx_t * 128 + nvals] = 1.0
    ST[idx_r, idx_ch, nvals, idx_t * 128 + idx_e] = 1.0

    # ---- L1 host tables ----
    W1 = _f32(inputs["W1"])
    h1 = x @ W1
    es1 = (h1.reshape(N, HH, C1) * _f32(inputs["as1"])[None]).sum(-1)
    ed1 = (h1.reshape(N, HH, C1) * _f32(inputs["ad1"])[None]).sum(-1)

    tab1 = np.zeros((RN, RW1), dtype=ml_dtypes.bfloat16)
    for c in range(R):
        rows = slice(c * NPAD, c * NPAD + NPC)
        tab1[rows, : x.shape[1]] = _bf(x[c * NPC : (c + 1) * NPC])
        tab1[rows, XW1 : XW1 + HH] = _bf(es1[c * NPC : (c + 1) * NPC])

    def chunk_layout(a, width):  # [NPC, w] -> [128, CH*w]
        f = np.zeros((NPAD, width), dtype=np.float32)
        f[: a.shape[0]] = a
        return _bf(f.reshape(CH, 128, width).transpose(1, 0, 2).reshape(128, CH * width))

    edt1 = np.stack([chunk_layout(ed1[c * NPC : (c + 1) * NPC], HH) for c in range(R)])

    B = 16
    cnt = np.zeros(B, dtype=np.float32)
    np.add.at(cnt, batch, 1.0)
    wp = 1.0 / np.maximum(cnt, 1.0)
    Ppool = []
    for c in range(R):
        P = np.zeros((NPC, B), dtype=np.float32)
        bloc = batch[c * NPC : (c + 1) * NPC]
        P[np.arange(NPC), bloc] = wp[bloc]
        Ppool.append(chunk_layout(P, B))
    Ppool = np.stack(Ppool)

    # ---- weights ----
    def pack_w(W, C, K):
        KS = (K + 127) // 128
        Wp = np.zeros((KS * 128, HH * C), dtype=np.float32)
        Wp[: W.shape[0]] = W
        out = np.zeros((128, HH * KS * C), dtype=ml_dtypes.bfloat16)
        for h in range(HH):
            for ks in range(KS):
                out[:, (h * KS + ks) * C : (h * KS + ks + 1) * C] = _bf(
                    Wp[ks * 128 : (ks + 1) * 128, h * C : (h + 1) * C]
                )
        return out, KS

    W2, W3 = _f32(inputs["W2"]), _f32(inputs["W3"])
    hw1, KS1 = pack_w(W1, C1, XW1)
    hw2, KS2 = pack_w(W2, C2, XW2)
    hw3, KS3 = pack_w(W3, C3, XW3)

    def pack_ae(W, a_s, a_d, C, K):
        KS = (K + 127) // 128
        Wae = np.zeros((KS * 128, 2 * HH), dtype=np.float32)
        Wr = W.reshape(K, HH, C)
        Wae[:K, :HH] = (Wr * a_s[None]).sum(-1)
        Wae[:K, HH:] = (Wr * a_d[None]).sum(-1)
        out = np.zeros((128, KS * 2 * HH), dtype=ml_dtypes.bfloat16)
        for ks in range(KS):
            out[:, ks * 2 * HH : (ks + 1) * 2 * HH] = _bf(Wae[ks * 128 : (ks + 1) * 128])
        return out

    hwae2 = pack_ae(W2, _f32(inputs["as2"]), _f32(inputs["ad2"]), C2, XW2)
    hwae3 = pack_ae(W3, _f32(inputs["as3"]), _f32(inputs["ad3"]), C3, XW3)

    hb1 = np.tile(_f32(inputs["b1"])[None, :], (128, 1))
    hb2 = np.tile(_f32(inputs["b2"])[None, :], (128, 1))
    hb3 = np.tile(_f32(inputs["b3"])[None, :], (128, 1))

    def pack_lhs(W):
        K, M = W.shape
        KS = (K + 127) // 128
        Wp = np.zeros((KS * 128, M), dtype=np.float32)
        Wp[:K] = W
        out = np.zeros((128, KS * M), dtype=ml_dtypes.bfloat16)
        for ks in range(KS):
            out[:, ks * M : (ks + 1) * M] = _bf(Wp[ks * 128 : (ks + 1) * 128])
        return out, KS, M

    hwa1, KA1, MA1 = pack_lhs(_f32(inputs["Wa1"]))
    hwa2, KA2, MA2 = pack_lhs(_f32(inputs["Wa2"]))
    hwa3, KA3, MA3 = pack_lhs(_f32(inputs["Wa3"]))
    hwc1, KC1, MC1 = pack_lhs(_f32(inputs["Wc1"]))
    hwc2, KC2, MC2 = pack_lhs(_f32(inputs["Wc2"]))
    hwc3, KC3, MC3 = pack_lhs(_f32(inputs["Wc3"]))

    def bias_cols(b):
        J = b.shape[0]
        n = max(1, (J + 127) // 128)
        bp = np.zeros(n * 128, dtype=np.float32)
        bp[:J] = b
        return np.ascontiguousarray(bp.reshape(n, 128).T)

    hba1 = bias_cols(_f32(inputs["ba1"]))
    hba2 = bias_cols(_f32(inputs["ba2"]))
    hba3 = bias_cols(_f32(inputs["ba3"]))
    hbc1 = bias_cols(_f32(inputs["bc1"]))
    hbc2 = bias_cols(_f32(inputs["bc2"]))
    hbc3 = bias_cols(_f32(inputs["bc3"]))

    cfg = dict(
        N=N, E=E, B=B, R=R, NPC=NPC, NPAD=NPAD, CH=CH, RN=RN, T=T, IW=IW,
        C1=C1, C2=C2, C3=C3, XW1=XW1, XW2=XW2, XW3=XW3,
        RW1=RW1, RW2=RW2, RW3=RW3, KS1=KS1, KS2=KS2, KS3=KS3,
        KA1=KA1, MA1=MA1, KA2=KA2, MA2=MA2, KA3=KA3, MA3=MA3,
        KC1=KC1, MC1=MC1, KC2=KC2, MC2=MC2, KC3=KC3, MC3=MC3,
    )
    shared = dict(
        tab1=np.ascontiguousarray(tab1),
        hw1=hw1, hw2=hw2, hw3=hw3, hwae2=hwae2, hwae3=hwae3,
        hb1=hb1, hb2=hb2, hb3=hb3,
        hwa1=hwa1, hwa2=hwa2, hwa3=hwa3, hwc1=hwc1, hwc2=hwc2, hwc3=hwc3,
        hba1=hba1, hba2=hba2, hba3=hba3, hbc1=hbc1, hbc2=hbc2, hbc3=hbc3,
    )
    in_maps = []
    for c in range(R):
        m = dict(shared)
        m["eidx"] = np.ascontiguousarray(ei16[c])
        m["Sh"] = np.ascontiguousarray(S[c])
        m["STh"] = np.ascontiguousarray(ST[c])
        m["edt1"] = np.ascontiguousarray(edt1[c])
        m["Ppool"] = np.ascontiguousarray(Ppool[c])
        in_maps.append(m)
    return cfg, in_maps


# --------------------------------------------------------------------------
# Device kernel builder
# --------------------------------------------------------------------------
def build_nc(cfg):
    R, CH, T, IW = cfg["R"], cfg["CH"], cfg["T"], cfg["IW"]
    NPAD, RN, B = cfg["NPAD"], cfg["RN"], cfg["B"]
    rg = [list(range(R))]

    nc = bacc.Bacc("TRN2", target_bir_lowering=False, debug=False, num_devices=R)

    tab1 = nc.dram_tensor("tab1", [RN, cfg["RW1"]], BF16, kind="ExternalInput")
    eidx = nc.dram_tensor("eidx", [CH, 128, IW], I16, kind="ExternalInput")
    Sh = nc.dram_tensor("Sh", [CH, 128, T * 128], BF16, kind="ExternalInput")
    STh = nc.dram_tensor("STh", [CH, 128, T * 128], BF16, kind="ExternalInput")
    edt1 = nc.dram_tensor("edt1", [128, CH * HH], BF16, kind="ExternalInput")
    Ppool = nc.dram_tensor("Ppool", [128, CH * B], BF16, kind="ExternalInput")
    ins = {}
    for nm, sh, dt in [
        ("hw1", [128, HH * cfg["KS1"] * cfg["C1"]], BF16),
        ("hw2", [128, HH * cfg["KS2"] * cfg["C2"]], BF16),
        ("hw3", [128, HH * cfg["KS3"] * cfg["C3"]], BF16),
        ("hwae2", [128, cfg["KS2"] * 2 * HH], BF16),
        ("hwae3", [128, cfg["KS3"] * 2 * HH], BF16),
        ("hb1", [128, HH * cfg["C1"]], F32),
        ("hb2", [128, HH * cfg["C2"]], F32),
        ("hb3", [128, HH * cfg["C3"]], F32),
        ("hwa1", [128, cfg["KA1"] * cfg["MA1"]], BF16),
        ("hwa2", [128, cfg["KA2"] * cfg["MA2"]], BF16),
        ("hwa3", [128, cfg["KA3"] * cfg["MA3"]], BF16),
        ("hwc1", [128, cfg["KC1"] * cfg["MC1"]], BF16),
        ("hwc2", [128, cfg["KC2"] * cfg["MC2"]], BF16),
        ("hwc3", [128, cfg["KC3"] * cfg["MC3"]], BF16),
        ("hba1", [128, max(1, cfg["MA1"] // 128)], F32),
        ("hba2", [128, max(1, cfg["MA2"] // 128)], F32),
        ("hba3", [128, 1], F32),
        ("hbc1", [128, max(1, cfg["MC1"] // 128)], F32),
        ("hbc2", [128, 1], F32),
        ("hbc3", [128, 1], F32),
    ]:
        ins[nm] = nc.dram_tensor(nm, sh, dt, kind="ExternalInput")

    out_lg = nc.dram_tensor("out_logitsT", [cfg["MA3"], 16], F32, kind="ExternalOutput")
    out_v = nc.dram_tensor("out_valueT", [1, 16], F32, kind="ExternalOutput")

    with tile.TileContext(nc) as tc:
        nc.gpsimd.load_library(library_config.mlp)
        with (
            tc.tile_pool(name="const", bufs=1) as cpool,
            tc.tile_pool(name="dram", bufs=1, space="DRAM") as dpool,
            tc.tile_pool(name="work", bufs=2) as wpool,
            tc.tile_pool(name="small", bufs=3) as spool,
            tc.tile_pool(name="psA", bufs=1, space="PSUM") as psA,
            tc.tile_pool(name="psB", bufs=2, space="PSUM") as psB,
            tc.tile_pool(name="psD", bufs=1, space="PSUM") as psD,
            tc.tile_pool(name="psP", bufs=1, space="PSUM") as psP,
        ):
            csb = {}
            for nm in ins:
                t = cpool.tile(ins[nm].shape, ins[nm].dtype, tag=nm)
                nc.sync.dma_start(t[:], ins[nm][:])
                csb[nm] = t
            ed_all = cpool.tile([128, CH * HH], BF16, tag="ed_all")
            nc.sync.dma_start(ed_all[:], edt1[:])
            Pp = cpool.tile([128, CH * B], BF16, tag="Pp")
            nc.sync.dma_start(Pp[:], Ppool[:])
            rec_all = cpool.tile([128, CH * HH], F32, tag="rec_all")

            tab2l = dpool.tile([NPAD, cfg["RW2"]], BF16, tag="t2l")
            tab2f = dpool.tile([RN, cfg["RW2"]], BF16, tag="t2f")
            PW = (HH * cfg["C3"] // 128) * 16
            tab3l = dpool.tile([NPAD, cfg["RW3"]], BF16, tag="t3l")
            tab3f = dpool.tile([RN, cfg["RW3"]], BF16, tag="t3f")
            aggd = dpool.tile([NPAD, HH * cfg["XW3"]], BF16, tag="aggd")
            poolb = dpool.tile([128, PW], F32, tag="poolb")
            poolr = dpool.tile([128, PW], F32, tag="poolr")

            poolT_ps = psP.tile([128, PW], F32, tag="poolT")
            SMW = max(T * HH, 128)

            def layer(XW, RW, C, KS, tab_src, hw, hb, tab_nxt, hwae, XW_nxt, RW_nxt=0):
                last = tab_nxt is None
                AGW = HH * XW
                OUTW = HH * C
                CW = min(XW, 128)
                # ------- phase A: edge aggregation -------
                for ch in range(CH):
                    idx = spool.tile([128, IW], I16, tag="idx")
                    nc.sync.dma_start(idx[:], eidx[ch, :, :])
                    Ssb = wpool.tile([128, T * 128], BF16, tag="Ssb")
                    nc.sync.dma_start(Ssb[:], Sh[ch, :, :])
                    STsb = wpool.tile([128, T * 128], BF16, tag="STsb")
                    nc.sync.dma_start(STsb[:], STh[ch, :, :])
                    xg = wpool.tile([128, T * RW], BF16, tag="xg")
                    for gs in range(0, T, 8):
                        nt = min(8, T - gs)
                        nc.gpsimd.dma_gather(
                            xg[:, gs * RW : (gs + nt) * RW].rearrange(
                                "p (t r) -> p t r", r=RW
                            ),
                            tab_src[:],
                            idx[:, gs * 8 : (gs + nt) * 8],
                            nt * 128,
                            nt * 128,
                            RW,
                        )
                    edp = psB.tile([128, SMW], F32, tag="sm")
                    for t in range(T):
                        nc.tensor.matmul(
                            edp[:, t * HH : (t + 1) * HH],
                            STsb[:, t * 128 : (t + 1) * 128],
                            ed_all[:, ch * HH : (ch + 1) * HH],
                            start=(t == 0),
                            stop=(t == T - 1),
                        )
                    lg = spool.tile([128, T * HH], F32, tag="lg")
                    nc.vector.tensor_add(
                        lg[:].rearrange("p (t h) -> p t h", h=HH),
                        xg[:].rearrange("p (t r) -> p t r", r=RW)[:, :, XW : XW + HH],
                        edp[:, 0 : T * HH].rearrange("p (t h) -> p t h", h=HH),
                    )
                    lg2 = spool.tile([128, T * HH], F32, tag="lg2")
                    nc.vector.tensor_scalar_mul(lg2[:], lg[:], NEG)
                    nc.vector.tensor_max(lg[:], lg[:], lg2[:])
                    wf = spool.tile([128, T * HH], F32, tag="wf")
                    nc.scalar.activation(wf[:], lg[:], AF.Exp)
                    wbf = spool.tile([128, T * HH], BF16, tag="wbf")
                    nc.vector.tensor_copy(wbf[:], wf[:])

                    agg = psA.tile([128, HH * 512], F32, tag="agg")
                    denp = psD.tile([128, HH], F32, tag="den")
                    for t in range(T):
                        Swt = spool.tile([128, HH * 128], BF16, tag="Sw")
                        for h in range(HH):
                            nc.vector.tensor_scalar_mul(
                                Swt[:, h * 128 : (h + 1) * 128],
                                Ssb[:, t * 128 : (t + 1) * 128],
                                wf[:, t * HH + h : t * HH + h + 1],
                            )
                        for h in range(HH):
                            nc.tensor.matmul(
                                agg[:, h * 512 : h * 512 + XW],
                                Swt[:, h * 128 : (h + 1) * 128],
                                xg[:, t * RW : t * RW + XW],
                                start=(t == 0),
                                stop=(t == T - 1),
                            )
                        nc.tensor.matmul(
                            denp[:],
                            Ssb[:, t * 128 : (t + 1) * 128],
                            wbf[:, t * HH : (t + 1) * HH],
                            start=(t == 0),
                            stop=(t == T - 1),
                        )
                    den = spool.tile([128, HH], F32, tag="den")
                    nc.vector.tensor_scalar_add(den[:], denp[:], 1e-16)
                    nc.vector.reciprocal(rec_all[:, ch * HH : (ch + 1) * HH], den[:])
                    agb = wpool.tile([128, AGW], BF16, tag="agb")
                    for h in range(HH):
                        eng = nc.scalar.copy if h % 2 == 0 else nc.vector.tensor_copy
                        eng(
                            agb[:, h * XW : (h + 1) * XW],
                            agg[:, h * 512 : h * 512 + XW],
                        )
                    nc.sync.dma_start(aggd[ch * 128 : (ch + 1) * 128, 0:AGW], agb[:])

                # ------- phase B: head matmul + next-layer prep -------
                for g in range(CH):
                    outp = psA.tile([128, HH * 512], F32, tag="agg")
                    for h in range(HH):
                        for ks in range(KS):
                            at = spool.tile([128, 128], BF16, tag="aT")
                            nc.sync.dma_start_transpose(
                                at[0:CW, :],
                                aggd[
                                    g * 128 : (g + 1) * 128,
                                    h * XW + ks * 128 : h * XW + ks * 128 + CW,
                                ],
                            )
                            nc.tensor.matmul(
                                outp[:, h * 512 : h * 512 + C],
                                at[0:CW, :],
                                hw[0:CW, (h * KS + ks) * C : (h * KS + ks + 1) * C],
                                start=(ks == 0),
                                stop=(ks == KS - 1),
                            )
                    outn = wpool.tile([128, OUTW], F32, tag="outn")
                    for h in range(HH):
                        nc.scalar.mul(
                            outn[:, h * C : (h + 1) * C],
                            outp[:, h * 512 : h * 512 + C],
                            rec_all[:, g * HH + h : g * HH + h + 1],
                        )
                    nc.vector.tensor_add(outn[:], outn[:], hb[:])
                    outb = wpool.tile([128, OUTW], BF16, tag="outb")
                    nc.vector.tensor_scalar_max(outb[:], outn[:], 0.0)
                    if last:
                        for fs in range(OUTW // 128):
                            nc.tensor.matmul(
                                poolT_ps[:, fs * 16 : fs * 16 + 16],
                                outb[:, fs * 128 : (fs + 1) * 128],
                                Pp[:, g * B : g * B + 16],
                                start=(g == 0 and fs == 0),
                                stop=(g == CH - 1 and fs == OUTW // 128 - 1),
                            )
                    else:
                        nc.sync.dma_start(
                            tab_nxt[g * 128 : (g + 1) * 128, 0:OUTW], outb[:]
                        )
                        esed = psB.tile([128, SMW], F32, tag="sm")
                        NFS = OUTW // 128
                        for fs in range(NFS):
                            xt = spool.tile([128, 128], BF16, tag="aT")
                            nc.sync.dma_start_transpose(
                                xt[:],
                                tab_nxt[
                                    g * 128 : (g + 1) * 128, fs * 128 : (fs + 1) * 128
                                ],
                            )
                            nc.tensor.matmul(
                                esed[:, 0 : 2 * HH],
                                xt[:],
                                hwae[:, fs * 2 * HH : (fs + 1) * 2 * HH],
                                start=(fs == 0),
                                stop=(fs == NFS - 1),
                            )
                        esb = spool.tile([128, RW_nxt - XW_nxt], BF16, tag="esb")
                        nc.vector.memset(esb[:], 0.0)
                        nc.vector.tensor_copy(esb[:, 0:HH], esed[:, 0:HH])
                        nc.sync.dma_start(
                            tab_nxt[g * 128 : (g + 1) * 128, XW_nxt:RW_nxt],
                            esb[:],
                        )
                        nc.vector.tensor_copy(
                            ed_all[:, g * HH : (g + 1) * HH], esed[:, HH : 2 * HH]
                        )

            layer(cfg["XW1"], cfg["RW1"], cfg["C1"], cfg["KS1"], tab1,
                  csb["hw1"], csb["hb1"], tab2l, csb["hwae2"], cfg["XW2"], cfg["RW2"])
            nc.gpsimd.collective_compute(
                "AllGather", mybir.AluOpType.bypass, replica_groups=rg,
                ins=[tab2l.opt()], outs=[tab2f.opt()],
            )
            layer(cfg["XW2"], cfg["RW2"], cfg["C2"], cfg["KS2"], tab2f,
                  csb["hw2"], csb["hb2"], tab3l, csb["hwae3"], cfg["XW3"], cfg["RW3"])
            nc.gpsimd.collective_compute(
                "AllGather", mybir.AluOpType.bypass, replica_groups=rg,
                ins=[tab3l.opt()], outs=[tab3f.opt()],
            )
            layer(cfg["XW3"], cfg["RW3"], cfg["C3"], cfg["KS3"], tab3f,
                  csb["hw3"], csb["hb3"], None, None, None)

            # ---- pool allreduce + heads ----
            psb_t = spool.tile([128, PW], F32, tag="poolsb")
            nc.vector.tensor_copy(psb_t[:], poolT_ps[:])
            nc.sync.dma_start(poolb[:], psb_t[:])
            nc.gpsimd.collective_compute(
                "AllReduce", mybir.AluOpType.add, replica_groups=rg,
                ins=[poolb.opt()], outs=[poolr.opt()],
            )
            gT = cpool.tile([128, PW], BF16, tag="gT")
            nc.gpsimd.dma_start(gT[:], poolr[:])  # f32 -> bf16 cast in DMA

            def mlp_layer(rhs, KB, M, wsb, bsb, outtag):
                MS = M // 128
                ps = psB.tile([128, SMW], F32, tag="sm")
                for ms in range(MS):
                    for ks in range(KB):
                        nc.tensor.matmul(
                            ps[:, ms * 16 : (ms + 1) * 16],
                            wsb[:, ks * M + ms * 128 : ks * M + ms * 128 + 128],
                            rhs[:, ks * 16 : (ks + 1) * 16],
                            start=(ks == 0 and ms == 0),
                            stop=(ks == KB - 1 and ms == MS - 1),
                        )
                ob = spool.tile([128, MS * 16], BF16, tag=outtag)
                for ms in range(MS):
                    nc.scalar.activation(
                        ob[:, ms * 16 : (ms + 1) * 16],
                        ps[:, ms * 16 : (ms + 1) * 16],
                        AF.Relu,
                        bias=bsb[:, ms : ms + 1],
                    )
                return ob

            a1 = mlp_layer(gT, cfg["KA1"], cfg["MA1"], csb["hwa1"], csb["hba1"], "a1")
            a2 = mlp_layer(a1, cfg["KA2"], cfg["MA2"], csb["hwa2"], csb["hba2"], "a2")
            lgp = psB.tile([128, SMW], F32, tag="sm")
            for ks in range(cfg["KA3"]):
                nc.tensor.matmul(
                    lgp[0 : cfg["MA3"], 0:16],
                    csb["hwa3"][:, ks * cfg["MA3"] : (ks + 1) * cfg["MA3"]],
                    a2[:, ks * 16 : (ks + 1) * 16],
                    start=(ks == 0),
                    stop=(ks == cfg["KA3"] - 1),
                )
            lgo = spool.tile([cfg["MA3"], 16], F32, tag="lgo")
            nc.scalar.activation(
                lgo[:], lgp[0 : cfg["MA3"], 0:16], AF.Tanh,
                bias=csb["hba3"][0 : cfg["MA3"], 0:1],
            )
            nc.sync.dma_start(out_lg[:], lgo[:])

            c1 = mlp_layer(gT, cfg["KC1"], cfg["MC1"], csb["hwc1"], csb["hbc1"], "c1")
            c2p = psB.tile([128, SMW], F32, tag="sm")
            nc.tensor.matmul(
                c2p[0 : cfg["MC2"], 0:16], csb["hwc2"][:, 0 : cfg["MC2"]], c1[:],
                start=True, stop=True,
            )
            c2 = spool.tile([cfg["MC2"], 16], BF16, tag="c2")
            nc.scalar.activation(
                c2[:], c2p[0 : cfg["MC2"], 0:16], AF.Relu,
                bias=csb["hbc2"][0 : cfg["MC2"], 0:1],
            )
            vp = psB.tile([128, SMW], F32, tag="sm")
            nc.tensor.matmul(
                vp[0:1, 0:16], csb["hwc3"][0 : cfg["MC2"], 0:1], c2[:],
                start=True, stop=True,
            )
            vo = spool.tile([1, 16], F32, tag="vo")
            nc.scalar.activation(
                vo[:], vp[0:1, 0:16], AF.Identity, bias=csb["hbc3"][0:1, 0:1]
            )
            nc.sync.dma_start(out_v[:], vo[:])

    nc.compile()
    return nc


# --------------------------------------------------------------------------
# Entry point
# --------------------------------------------------------------------------
def kernel(**inputs):
    import os
    import time
    from concourse.bass_utils import run_bass_kernel_spmd

    t0 = time.time()
    cfg, in_maps = preprocess(inputs, R=8)
    print(f"[kernel] preprocess {time.time()-t0:.1f}s T={cfg['T']}", flush=True)
    t1 = time.time()
    nc = build_nc(cfg)
    print(f"[kernel] build+compile {time.time()-t1:.1f}s", flush=True)
    trace = bool(int(os.environ.get("BASS_GNN_TRACE", "0")))
    t2 = time.time()
    res = run_bass_kernel_spmd(nc, in_maps, core_ids=list(range(8)), trace=trace)
    print(f"[kernel] run {time.time()-t2:.1f}s", flush=True)
    if trace and res.exec_time_ns is not None:
        print(f"HW exec time: {res.exec_time_ns} ns", flush=True)
    out = res.results[0]
    logits = np.ascontiguousarray(
        out["out_logitsT"].T.reshape(16, 40, 2), dtype=np.float32
    )
    value = np.ascontiguousarray(out["out_valueT"].T, dtype=np.float32)
    return logits, value
